# revision 25
# baseline (speedup 1.0000x reference)
"""Trainium2 Bass kernel for nn_FAttention1d (attention with softmax over the
QUERY axis).

Reference computation (B=2, H=16, S=2048, D=64, fp32):
    att[b,h,q,k] = sum_d qry[b,h,q,d] * key[b,h,k,d]
    att += reg * I_S                      (diagonal in (q,k))
    att = softmax(att, axis=q)            (normalize over the QUERY axis)
    out[b,h,q,v] = sum_k att[b,h,q,k] * val[b,h,k,v]

Sharding: the 32 (b,h) pairs are split 4-per-core across 8 NeuronCores; the
whole S=2048 attention chain is local to a core.

Device-side layout: compute S^T = K @ Q^T with k on the partition axis, so the
q-axis softmax is a free-axis reduction (fused into the exp pass via the ACT
accumulator), and exp(S^T) tiles feed the A^T V matmul directly as the moving
operand:
    out^T[v,q] = sum_k (val[k,v] / r[k])_stationary @ exp(S^T)[k,q]
with r[k] = sum_q exp(S^T[k,q]) folded into the val rows.

Engine balance (per core ~16.8M exp elements move PSUM->SBUF):
  - ACT: all exp ops. A minority of (head, k-tile) score tiles ("direct") are
    exp'd straight from PSUM in two [128,1024] ops; the rest are staged to
    SBUF by DVE and exp'd in one wide [128,2048] op (cheaper per column for
    ACT, costs DVE a copy). The direct count balances ACT vs DVE.
  - DVE: staging copies, PSUM diag adds for direct tiles, reciprocal, out^T
    PSUM->SBUF copies.
  - Pool: diag adds on staged SBUF tiles, r half merges, val/r scaling.
  - PE: QK^T and AV matmuls, AV drained between QK chunks so PE follows the
    ACT/DVE pacemakers without bursts.
PSUM: out^T accumulator [128,2048] (4 banks) + 2x [128,1024] score tiles.
"""

import numpy as np
from collections import deque
from contextlib import ExitStack

import concourse.bass as bass
import concourse.mybir as mybir
import concourse.tile as tile
from concourse import bacc
from concourse.bass_utils import run_bass_kernel_spmd

B, H, S, D = 2, 16, 2048, 64
N_CORES = 8
BH = B * H                     # 32
BH_PER_CORE = BH // N_CORES    # 4
NT = S // 128                  # 16 k-tiles of 128
F32 = mybir.dt.float32
F16 = mybir.dt.float16
BF16 = mybir.dt.bfloat16

# Some (pair, k-tile, head) score tiles are exp'd directly from PSUM (2x
# [128,1024] ACT ops); the rest are DVE-staged to SBUF and exp'd in one
# [128,2048] ACT op (cheaper per column for ACT, costs DVE the copies).
# Direct tiles are head-A-only on every other k-tile: each PSUM score
# buffer's consumer is then a single ~1.2us op (one exp or one copy), which
# fits the reuse window of the two-buffer PSUM ping-pong — a fully-direct
# tile's 2.5us ACT drain stalls PE and starves DVE. 16 direct of 64
# balances ACT (~139us) against DVE (~126us) with Pool well under both.
DIRECT_NS = set()


def _is_direct(p, n, s):
    return s == 0 and n in DIRECT_NS


# "X" score tiles stage only the h0 half (one DVE copy); the h1 half is
# exp'd straight from PSUM. Both exps are [128,1024] ops, short enough to
# meet the PSUM ping-pong reuse window even behind one queued wide exp.
# Trades ~1.2us DVE for ~0.4us ACT per tile to balance the two pacemakers.
X_NS = set()


def _is_x(p, n, s):
    return n in X_NS


def _build_kernel(nc, tc, ctx, qt, kt, vs, rg, ot):
    const_pool = ctx.enter_context(tc.tile_pool(name="const", bufs=1))
    q_pool = ctx.enter_context(tc.tile_pool(name="q", bufs=2))
    k_pool = ctx.enter_context(tc.tile_pool(name="k", bufs=2))
    v_pool = ctx.enter_context(tc.tile_pool(name="v", bufs=2))
    e_pool = ctx.enter_context(tc.tile_pool(name="e", bufs=8))
    stg_pool = ctx.enter_context(tc.tile_pool(name="stg", bufs=3))
    r_pool = ctx.enter_context(tc.tile_pool(name="r", bufs=2))
    vsc_pool = ctx.enter_context(tc.tile_pool(name="vsc", bufs=6))
    osb_pool = ctx.enter_context(tc.tile_pool(name="osb", bufs=2))
    st_pool = ctx.enter_context(tc.tile_pool(name="st", bufs=2, space="PSUM"))
    o_pool = ctx.enter_context(tc.tile_pool(name="o", bufs=1, space="PSUM"))

    rg_eye = const_pool.tile([128, 128], F32)
    nc.sync.dma_start(rg_eye[:], rg[:])
    # warm the ACT Exp table during the input DMA so the first real exp
    # doesn't pay the 1.3us table load
    warm = const_pool.tile([128, 1], F32)
    nc.scalar.activation(warm[:], rg_eye[:, 0:1],
                         mybir.ActivationFunctionType.Exp)

    AB = (0, 1)
    for p in range(BH_PER_CORE // 2):
        bh = (2 * p, 2 * p + 1)
        q2 = q_pool.tile([128, S], F16, tag="q2", name="q2")
        k2 = k_pool.tile([128, S], F16, tag="k2", name="k2")
        # split input DMAs across both HWDGE queues so the first QK chunk
        # can start early
        nc.scalar.dma_start(k2[:, 0:128], kt[p][:, 0:128])
        nc.sync.dma_start(q2[:, 0:512], qt[p][:, 0:512])
        nc.sync.dma_start(q2[:, 512:1024], qt[p][:, 512:1024])
        nc.scalar.dma_start(k2[:, 128:], kt[p][:, 128:])
        nc.sync.dma_start(q2[:, 1024:], qt[p][:, 1024:])
        v_sb = [None, None]
        for s in AB:
            v_sb[s] = v_pool.tile([128, NT * 64], F32, tag=f"v{s}", name=f"v_sb{s}")
            nc.scalar.dma_start(v_sb[s][:], vs[bh[s]])

        # out^T for the pair: partitions 0-63 = bh A, 64-127 = bh B
        o_ps = o_pool.tile([128, S], F32)
        r_all = [r_pool.tile([128, 2, NT], F32, tag=f"rall{s}", name=f"r_all{s}") for s in AB]
        r_sum = [r_pool.tile([128, NT], F32, tag=f"rsum{s}", name=f"r_sum{s}") for s in AB]
        e_tiles = [[None] * NT, [None] * NT]
        vsc_tiles = [[None] * NT, [None] * NT]
        pending = deque()

        def queue_av_tiles(ms):
            # enqueue col-packed AV matmuls for k-tiles ms; drained a few at a
            # time between QK chunks so PE tracks the ACT/DVE pacemakers
            for m in ms:
                for ch in range(4):
                    pending.append((m, ch))

        out_sb = osb_pool.tile([128, S], F32)

        def drain_pending(k=2):
            for _ in range(k):
                if not pending:
                    return
                m, h = pending.popleft()
                ch = slice(h * 512, (h + 1) * 512)
                for s in AB:
                    # bh A -> out partitions 0-63, bh B -> 64-127
                    nc.tensor.matmul(
                        o_ps[64 * s:64 * s + 64, ch],
                        lhsT=vsc_tiles[s][m][:],
                        rhs=e_tiles[s][m][:, ch],
                        start=(m == 0),
                        stop=(m == NT - 1),
                        skip_group_check=True,
                    )
                if m == NT - 1:
                    # last accumulation for this q-chunk: evacuate + ship now
                    # so copies/DMAs overlap the remaining AV drains. The copy
                    # runs on ACT (same act table as Exp, no table switch) to
                    # keep DVE, the staging pacemaker, free.
                    nc.scalar.copy(out_sb[:, ch], o_ps[:, ch])
                    for s in AB:
                        nc.scalar.dma_start(ot[bh[s]][:, ch],
                                            out_sb[64 * s:64 * s + 64, ch])

        def emit_vsc(ms):
            # vsc[m] = val / r[m] in one Pool op (normalize_recip fuses the
            # reciprocal); keeps the r chain off DVE so staging copies never
            # head-of-line block on a reduction
            for m in ms:
                for s in AB:
                    vsc = vsc_pool.tile([128, 64], BF16, tag=f"vsc{s}",
                                        name=f"vsc{s}")
                    vsc_tiles[s][m] = vsc
                    nc.gpsimd.normalize_recip(
                        vsc[:], v_sb[s][:, m * 64:(m + 1) * 64],
                        r_sum[s][:, m:m + 1]
                    )

        # vsc for tile m is emitted inside tile VSC_AT[m] (after that tile's
        # diag units, so Pool never gates the diag->exp chain); its AV
        # matmuls are queued one tile later so PE never waits on fresh vsc.
        VSC_AT = {4: [0, 1, 2, 3], 8: [4, 5, 6, 7], 12: [8, 9, 10, 11],
                  14: [12], 15: [13]}
        QUEUE_AT = {5: [0, 1, 2, 3], 9: [4, 5, 6, 7], 13: [8, 9, 10, 11],
                    15: [12]}

        def emit_r_merge_and_vsc(ms):
            for m in ms:
                for s in AB:
                    if _is_direct(p, m, s) or _is_x(p, m, s):
                        nc.gpsimd.tensor_add(
                            r_sum[s][:, m:m + 1], r_all[s][:, 0, m:m + 1],
                            r_all[s][:, 1, m:m + 1]
                        )
            emit_vsc(ms)

        for n in range(NT):
            hd = n // 8               # q-half containing this tile's diagonal
            cd = (n % 8) * 128        # diag column offset within that half
            queue_av_tiles(QUEUE_AT.get(n, []))
            for s in AB:
                e_tiles[s][n] = e_pool.tile([128, S], BF16, tag=f"e{s}",
                                            name=f"e{s}_{n}")
            stage = [None if _is_direct(p, n, s) else
                     stg_pool.tile(
                         [128, 1024 if _is_x(p, n, s) else S], F32,
                         tag=f"stg{s}" + ("x" if _is_x(p, n, s) else ""),
                         name=f"stage{s}")
                     for s in AB]
            for h in range(2):
                for s in AB:
                    direct = _is_direct(p, n, s)
                    st = st_pool.tile([128, 1024], F32)
                    for j in range(2):
                        q0 = h * 1024 + j * 512
                        nc.tensor.matmul(
                            st[:, j * 512:(j + 1) * 512],
                            lhsT=k2[64 * s:64 * s + 64, n * 128:(n + 1) * 128],
                            rhs=q2[64 * s:64 * s + 64, q0:q0 + 512],
                            start=True,
                            stop=True,
                        )
                    if direct:
                        if h == hd:
                            nc.vector.tensor_add(
                                st[:, cd:cd + 128], st[:, cd:cd + 128],
                                rg_eye[:])
                        nc.scalar.activation(
                            e_tiles[s][n][:, h * 1024:(h + 1) * 1024],
                            st[:],
                            mybir.ActivationFunctionType.Exp,
                            accum_out=r_all[s][:, h:h + 1, n:n + 1],
                        )
                    elif _is_x(p, n, s):
                        if h == 0:
                            nc.vector.tensor_copy(
                                stage[s][:, 0:1024], st[:])
                            if hd == 0:
                                nc.gpsimd.tensor_add(
                                    stage[s][:, cd:cd + 128],
                                    stage[s][:, cd:cd + 128],
                                    rg_eye[:])
                        else:
                            if hd == 1:
                                nc.vector.tensor_add(
                                    st[:, cd:cd + 128], st[:, cd:cd + 128],
                                    rg_eye[:])
                            nc.scalar.activation(
                                e_tiles[s][n][:, 0:1024],
                                stage[s][:, 0:1024],
                                mybir.ActivationFunctionType.Exp,
                                accum_out=r_all[s][:, 0:1, n:n + 1],
                            )
                            nc.scalar.activation(
                                e_tiles[s][n][:, 1024:2048],
                                st[:],
                                mybir.ActivationFunctionType.Exp,
                                accum_out=r_all[s][:, 1:2, n:n + 1],
                            )
                    else:
                        nc.vector.tensor_copy(
                            stage[s][:, h * 1024:(h + 1) * 1024], st[:])
                        if h == hd:
                            # diag add on the SBUF stage, off the PSUM path
                            nc.gpsimd.tensor_add(
                                stage[s][:, n * 128:(n + 1) * 128],
                                stage[s][:, n * 128:(n + 1) * 128],
                                rg_eye[:])
                        if h == 1:
                            nc.scalar.activation(
                                e_tiles[s][n][:],
                                stage[s][:],
                                mybir.ActivationFunctionType.Exp,
                                accum_out=r_sum[s][:, n:n + 1],
                            )
                    if s == 1 and h == hd:
                        emit_r_merge_and_vsc(VSC_AT.get(n, []))
                    drain_pending(1 if n < 6 else (2 if n < 13 else 3))
        emit_r_merge_and_vsc([14, 15])
        queue_av_tiles([13, 14, 15])
        while pending:
            drain_pending(4)


_NC_CACHE = {}


def build_nc(repeats=1):
    key = repeats
    if key in _NC_CACHE:
        return _NC_CACHE[key]
    nc = bacc.Bacc("TRN2", target_bir_lowering=False, debug=False)
    qt = nc.dram_tensor("qt", [BH_PER_CORE // 2, 2 * D, S], F16, kind="ExternalInput").ap()
    kt = nc.dram_tensor("kt", [BH_PER_CORE // 2, 2 * D, S], F16, kind="ExternalInput").ap()
    vs = nc.dram_tensor("vs", [BH_PER_CORE, 128, NT * 64], F32, kind="ExternalInput").ap()
    rg = nc.dram_tensor("rg", [128, 128], F32, kind="ExternalInput").ap()
    ot = nc.dram_tensor("ot", [BH_PER_CORE, D, S], F32, kind="ExternalOutput").ap()
    with tile.TileContext(nc) as tc, ExitStack() as ctx:
        if repeats == 1:
            _build_kernel(nc, tc, ctx, qt, kt, vs, rg, ot)
        else:
            # benchmarking mode: repeat the whole kernel body in an on-device
            # loop so per-iteration time can be extracted from wall clock
            with tc.For_i(0, repeats, 1,
                          hint_engines=(mybir.EngineType.PE,
                                        mybir.EngineType.Activation,
                                        mybir.EngineType.DVE)):
                _build_kernel(nc, tc, ctx, qt, kt, vs, rg, ot)
    nc.compile()
    _NC_CACHE[key] = nc
    return nc


def _prep_inputs(qry, key, val, reg):
    """Host-side shard + layout prep. Returns per-core input maps."""
    q = np.ascontiguousarray(np.asarray(qry, dtype=np.float32)).reshape(BH, S, D)
    k = np.ascontiguousarray(np.asarray(key, dtype=np.float32)).reshape(BH, S, D)
    v = np.ascontiguousarray(np.asarray(val, dtype=np.float32)).reshape(BH, S, D)
    rg = (np.eye(128, dtype=np.float32) * np.float32(np.asarray(reg)))

    in_maps = []
    for c in range(N_CORES):
        sl = slice(c * BH_PER_CORE, (c + 1) * BH_PER_CORE)
        qt = np.ascontiguousarray(
            q[sl].transpose(0, 2, 1).reshape(BH_PER_CORE // 2, 2 * D, S)
        ).astype(np.float16)                                          # [2, 128, S]
        kt = np.ascontiguousarray(
            k[sl].transpose(0, 2, 1).reshape(BH_PER_CORE // 2, 2 * D, S)
        ).astype(np.float16)                                          # [2, 128, S]
        vv = v[sl].reshape(BH_PER_CORE, NT, 128, D)
        vs = np.ascontiguousarray(vv.transpose(0, 2, 1, 3)).reshape(
            BH_PER_CORE, 128, NT * D)                                 # [4, 128, 1024]
        in_maps.append({"qt": qt, "kt": kt, "vs": vs, "rg": rg})
    return in_maps


def kernel(qry, key, val, reg):
    nc = build_nc()
    in_maps = _prep_inputs(qry, key, val, reg)
    res = run_bass_kernel_spmd(nc, in_maps, list(range(N_CORES)))
    out = np.empty((BH, S, D), dtype=np.float32)
    for c in range(N_CORES):
        ot = res.results[c]["ot"]                                    # [4, 64, S]
        for i in range(BH_PER_CORE):
            out[c * BH_PER_CORE + i] = ot[i].T
    return out.reshape(B, H, S, D)


# revision 37
# speedup vs baseline: 2.9901x; 2.9901x over previous
"""Trainium2 Bass kernel for nn_FAttention1d (attention with softmax over the
QUERY axis).

Reference computation (B=2, H=16, S=2048, D=64, fp32):
    att[b,h,q,k] = sum_d qry[b,h,q,d] * key[b,h,k,d]
    att += reg * I_S                      (diagonal in (q,k))
    att = softmax(att, axis=q)            (normalize over the QUERY axis)
    out[b,h,q,v] = sum_k att[b,h,q,k] * val[b,h,k,v]

Sharding: the 32 (b,h) pairs are split 4-per-core across 8 NeuronCores; the
whole S=2048 attention chain is local to a core.

Device-side layout: compute S^T = K @ Q^T with k on the partition axis, so the
q-axis softmax is a free-axis reduction (fused into the exp pass via the ACT
accumulator), and exp(S^T) tiles feed the A^T V matmul directly as the moving
operand:
    out^T[v,q] = sum_k (val[k,v] / r[k])_stationary @ exp(S^T)[k,q]
with r[k] = sum_q exp(S^T[k,q]) folded into the val rows.

Engine balance (per core ~16.8M exp elements move PSUM->SBUF):
  - ACT: all exp ops. A minority of (head, k-tile) score tiles ("direct") are
    exp'd straight from PSUM in two [128,1024] ops; the rest are staged to
    SBUF by DVE and exp'd in one wide [128,2048] op (cheaper per column for
    ACT, costs DVE a copy). The direct count balances ACT vs DVE.
  - DVE: staging copies, PSUM diag adds for direct tiles, reciprocal, out^T
    PSUM->SBUF copies.
  - Pool: diag adds on staged SBUF tiles, r half merges, val/r scaling.
  - PE: QK^T and AV matmuls, AV drained between QK chunks so PE follows the
    ACT/DVE pacemakers without bursts.
PSUM: out^T accumulator [128,2048] (4 banks) + 2x [128,1024] score tiles.
"""

import numpy as np
from collections import deque
from contextlib import ExitStack

import concourse.bass as bass
import concourse.mybir as mybir
import concourse.tile as tile
from concourse import bacc
from concourse.bass_utils import run_bass_kernel_spmd

B, H, S, D = 2, 16, 2048, 64
N_CORES = 8
BH = B * H                     # 32
BH_PER_CORE = BH // N_CORES    # 4
NT = S // 128                  # 16 k-tiles of 128
F32 = mybir.dt.float32
F16 = mybir.dt.float16
BF16 = mybir.dt.bfloat16

# Some (pair, k-tile, head) score tiles are exp'd directly from PSUM (2x
# [128,1024] ACT ops); the rest are DVE-staged to SBUF and exp'd in one
# [128,2048] ACT op (cheaper per column for ACT, costs DVE the copies).
# Direct tiles are head-A-only on every other k-tile: each PSUM score
# buffer's consumer is then a single ~1.2us op (one exp or one copy), which
# fits the reuse window of the two-buffer PSUM ping-pong — a fully-direct
# tile's 2.5us ACT drain stalls PE and starves DVE. 16 direct of 64
# balances ACT (~139us) against DVE (~126us) with Pool well under both.
DIRECT_NS = set()


def _is_direct(p, n, s):
    return s == 0 and n in DIRECT_NS


# "X" score tiles stage only the h0 half (one DVE copy); the h1 half is
# exp'd straight from PSUM. Both exps are [128,1024] ops, short enough to
# meet the PSUM ping-pong reuse window even behind one queued wide exp.
# Trades ~1.2us DVE for ~0.4us ACT per tile to balance the two pacemakers.
X_NS = set()


def _is_x(p, n, s):
    return n in X_NS


def _build_kernel(nc, tc, ctx, qt, kt, vs, rg, rgb, ot):
    const_pool = ctx.enter_context(tc.tile_pool(name="const", bufs=1))
    q_pool = ctx.enter_context(tc.tile_pool(name="q", bufs=2))
    k_pool = ctx.enter_context(tc.tile_pool(name="k", bufs=2))
    v_pool = ctx.enter_context(tc.tile_pool(name="v", bufs=2))
    e_pool = ctx.enter_context(tc.tile_pool(name="e", bufs=8))
    stg_pool = ctx.enter_context(tc.tile_pool(name="stg", bufs=3))
    r_pool = ctx.enter_context(tc.tile_pool(name="r", bufs=2))
    vsc_pool = ctx.enter_context(tc.tile_pool(name="vsc", bufs=6))
    osb_pool = ctx.enter_context(tc.tile_pool(name="osb", bufs=2))
    st_pool = ctx.enter_context(tc.tile_pool(name="st", bufs=2, space="PSUM"))
    o_pool = ctx.enter_context(tc.tile_pool(name="o", bufs=1, space="PSUM"))

    rg_eye = const_pool.tile([128, 128], F32)
    nc.sync.dma_start(rg_eye[:], rg[:])
    # [I, reg*I] as bf16 for the PE diag-accumulate matmul (I.T @ reg*I adds
    # reg to the score diagonal in PSUM, costing PE ~60ns instead of a DVE op)
    eyb = const_pool.tile([128, 256], BF16)
    nc.sync.dma_start(eyb[:], rgb[:])
    # warm the ACT Exp table during the input DMA so the first real exp
    # doesn't pay the 1.3us table load
    warm = const_pool.tile([128, 1], F32)
    nc.scalar.activation(warm[:], rg_eye[:, 0:1],
                         mybir.ActivationFunctionType.Exp)

    AB = (0, 1)
    for p in range(BH_PER_CORE // 2):
        bh = (2 * p, 2 * p + 1)
        q2 = q_pool.tile([128, S], F16, tag="q2", name="q2")
        k2 = k_pool.tile([128, S], F16, tag="k2", name="k2")
        # split input DMAs across both HWDGE queues so the first QK chunk
        # can start early
        nc.scalar.dma_start(k2[:, 0:128], kt[p][:, 0:128])
        nc.sync.dma_start(q2[:, 0:512], qt[p][:, 0:512])
        nc.sync.dma_start(q2[:, 512:1024], qt[p][:, 512:1024])
        nc.scalar.dma_start(k2[:, 128:], kt[p][:, 128:])
        nc.sync.dma_start(q2[:, 1024:], qt[p][:, 1024:])
        v_sb = [None, None]
        for s in AB:
            v_sb[s] = v_pool.tile([128, NT * 64], F32, tag=f"v{s}", name=f"v_sb{s}")
            nc.scalar.dma_start(v_sb[s][:], vs[bh[s]])

        # out^T for the pair: partitions 0-63 = bh A, 64-127 = bh B
        o_ps = o_pool.tile([128, S], F32)
        r_all = [r_pool.tile([128, 2, NT], F32, tag=f"rall{s}", name=f"r_all{s}") for s in AB]
        r_sum = [r_pool.tile([128, NT], F32, tag=f"rsum{s}", name=f"r_sum{s}") for s in AB]
        r_inv = [r_pool.tile([128, NT], F32, tag=f"rinv{s}", name=f"r_inv{s}") for s in AB]
        e_tiles = [[None] * NT, [None] * NT]
        vsc_tiles = [[None] * NT, [None] * NT]
        pending = deque()

        def queue_av_tiles(ms):
            # enqueue col-packed AV matmuls for k-tiles ms; drained a few at a
            # time between QK chunks so PE tracks the ACT/DVE pacemakers
            for m in ms:
                for ch in range(4):
                    pending.append((m, ch))

        out_sb = osb_pool.tile([128, S], F32)

        def drain_pending(k=2):
            for _ in range(k):
                if not pending:
                    return
                m, h = pending.popleft()
                ch = slice(h * 512, (h + 1) * 512)
                for s in AB:
                    # bh A -> out partitions 0-63, bh B -> 64-127
                    nc.tensor.matmul(
                        o_ps[64 * s:64 * s + 64, ch],
                        lhsT=vsc_tiles[s][m][:],
                        rhs=e_tiles[s][m][:, ch],
                        start=(m == 0),
                        stop=(m == NT - 1),
                        skip_group_check=True,
                    )
                if m == NT - 1:
                    # last accumulation for this q-chunk: evacuate + ship now
                    # so copies/DMAs overlap the remaining AV drains. The copy
                    # runs on ACT (same act table as Exp, no table switch) to
                    # keep DVE, the staging pacemaker, free.
                    nc.scalar.copy(out_sb[:, ch], o_ps[:, ch])
                    for s in AB:
                        nc.scalar.dma_start(ot[bh[s]][:, ch],
                                            out_sb[64 * s:64 * s + 64, ch])

        def emit_vsc(ms):
            # r_inv on DVE (cheap [128,few] op); vsc[m] = val * r_inv[m] on
            # ACT as a Copy with a per-partition scale (same act table as
            # Exp, no table switch). Pool/gpsimd compute ops are avoided
            # entirely: their real Q7 launch overhead is micro-seconds, not
            # the ~100ns the cost model charges.
            if not ms:
                return
            for s in AB:
                nc.vector.reciprocal_approx_fast(
                    r_inv[s][:, ms[0]:ms[-1] + 1],
                    r_sum[s][:, ms[0]:ms[-1] + 1])
            for m in ms:
                for s in AB:
                    vsc = vsc_pool.tile([128, 64], BF16, tag=f"vsc{s}",
                                        name=f"vsc{s}")
                    vsc_tiles[s][m] = vsc
                    nc.vector.tensor_scalar_mul(
                        vsc[:], v_sb[s][:, m * 64:(m + 1) * 64],
                        r_inv[s][:, m:m + 1],
                    )

        # vsc for tile m is emitted inside tile VSC_AT[m] (after that tile's
        # diag units, so Pool never gates the diag->exp chain); its AV
        # matmuls are queued one tile later so PE never waits on fresh vsc.
        VSC_AT = {4: [0, 1, 2, 3], 8: [4, 5, 6, 7], 12: [8, 9, 10, 11],
                  14: [12], 15: [13]}
        QUEUE_AT = {5: [0, 1, 2, 3], 9: [4, 5, 6, 7], 13: [8, 9, 10, 11],
                    15: [12]}

        def emit_r_merge_and_vsc(ms):
            for m in ms:
                for s in AB:
                    if _is_direct(p, m, s) or _is_x(p, m, s):
                        nc.vector.tensor_add(
                            r_sum[s][:, m:m + 1], r_all[s][:, 0, m:m + 1],
                            r_all[s][:, 1, m:m + 1]
                        )
            emit_vsc(ms)

        for n in range(NT):
            hd = n // 8               # q-half containing this tile's diagonal
            cd = (n % 8) * 128        # diag column offset within that half
            queue_av_tiles(QUEUE_AT.get(n, []))
            for s in AB:
                e_tiles[s][n] = e_pool.tile([128, S], BF16, tag=f"e{s}",
                                            name=f"e{s}_{n}")
            stage = [None if _is_direct(p, n, s) else
                     stg_pool.tile(
                         [128, 1024 if _is_x(p, n, s) else S], F32,
                         tag=f"stg{s}" + ("x" if _is_x(p, n, s) else ""),
                         name=f"stage{s}")
                     for s in AB]
            for h in range(2):
                for s in AB:
                    direct = _is_direct(p, n, s)
                    st = st_pool.tile([128, 1024], F32)
                    for j in range(2):
                        q0 = h * 1024 + j * 512
                        nc.tensor.matmul(
                            st[:, j * 512:(j + 1) * 512],
                            lhsT=k2[64 * s:64 * s + 64, n * 128:(n + 1) * 128],
                            rhs=q2[64 * s:64 * s + 64, q0:q0 + 512],
                            start=True,
                            stop=True,
                        )
                    if direct:
                        if h == hd:
                            nc.vector.tensor_add(
                                st[:, cd:cd + 128], st[:, cd:cd + 128],
                                rg_eye[:])
                        nc.scalar.activation(
                            e_tiles[s][n][:, h * 1024:(h + 1) * 1024],
                            st[:],
                            mybir.ActivationFunctionType.Exp,
                            accum_out=r_all[s][:, h:h + 1, n:n + 1],
                        )
                    elif _is_x(p, n, s):
                        if h == 0:
                            nc.vector.tensor_copy(
                                stage[s][:, 0:1024], st[:])
                            if hd == 0:
                                nc.vector.tensor_add(
                                    stage[s][:, cd:cd + 128],
                                    stage[s][:, cd:cd + 128],
                                    rg_eye[:])
                        else:
                            if hd == 1:
                                nc.vector.tensor_add(
                                    st[:, cd:cd + 128], st[:, cd:cd + 128],
                                    rg_eye[:])
                            nc.scalar.activation(
                                e_tiles[s][n][:, 0:1024],
                                stage[s][:, 0:1024],
                                mybir.ActivationFunctionType.Exp,
                                accum_out=r_all[s][:, 0:1, n:n + 1],
                            )
                            nc.scalar.activation(
                                e_tiles[s][n][:, 1024:2048],
                                st[:],
                                mybir.ActivationFunctionType.Exp,
                                accum_out=r_all[s][:, 1:2, n:n + 1],
                            )
                    else:
                        if h == hd:
                            # diag add in PSUM on PE: st[:, cd:] += I.T @ reg*I
                            nc.tensor.matmul(
                                st[:, cd:cd + 128],
                                lhsT=eyb[:, 0:128],
                                rhs=eyb[:, 128:256],
                                start=False,
                                stop=True,
                                skip_group_check=True,
                            )
                        nc.vector.tensor_copy(
                            stage[s][:, h * 1024:(h + 1) * 1024], st[:])
                        if h == 1:
                            nc.scalar.activation(
                                e_tiles[s][n][:],
                                stage[s][:],
                                mybir.ActivationFunctionType.Exp,
                                accum_out=r_sum[s][:, n:n + 1],
                            )
                    if s == 1 and h == hd:
                        emit_r_merge_and_vsc(VSC_AT.get(n, []))
                    drain_pending(1 if n < 6 else (2 if n < 13 else 3))
        emit_r_merge_and_vsc([14, 15])
        queue_av_tiles([13, 14, 15])
        while pending:
            drain_pending(4)


_NC_CACHE = {}


def build_nc(repeats=1):
    key = repeats
    if key in _NC_CACHE:
        return _NC_CACHE[key]
    nc = bacc.Bacc("TRN2", target_bir_lowering=False, debug=False)
    qt = nc.dram_tensor("qt", [BH_PER_CORE // 2, 2 * D, S], F16, kind="ExternalInput").ap()
    kt = nc.dram_tensor("kt", [BH_PER_CORE // 2, 2 * D, S], F16, kind="ExternalInput").ap()
    vs = nc.dram_tensor("vs", [BH_PER_CORE, 128, NT * 64], F32, kind="ExternalInput").ap()
    rg = nc.dram_tensor("rg", [128, 128], F32, kind="ExternalInput").ap()
    rgb = nc.dram_tensor("rgb", [128, 256], BF16, kind="ExternalInput").ap()
    ot = nc.dram_tensor("ot", [BH_PER_CORE, D, S], F32, kind="ExternalOutput").ap()
    with tile.TileContext(nc) as tc, ExitStack() as ctx:
        if repeats == 1:
            _build_kernel(nc, tc, ctx, qt, kt, vs, rg, rgb, ot)
        else:
            # benchmarking mode: repeat the whole kernel body in an on-device
            # loop so per-iteration time can be extracted from wall clock
            with tc.For_i(0, repeats, 1,
                          hint_engines=(mybir.EngineType.PE,
                                        mybir.EngineType.Activation,
                                        mybir.EngineType.DVE)):
                _build_kernel(nc, tc, ctx, qt, kt, vs, rg, rgb, ot)
    nc.compile()
    _NC_CACHE[key] = nc
    return nc


def _prep_inputs(qry, key, val, reg):
    """Host-side shard + layout prep. Returns per-core input maps."""
    q = np.ascontiguousarray(np.asarray(qry, dtype=np.float32)).reshape(BH, S, D)
    k = np.ascontiguousarray(np.asarray(key, dtype=np.float32)).reshape(BH, S, D)
    v = np.ascontiguousarray(np.asarray(val, dtype=np.float32)).reshape(BH, S, D)
    rg = (np.eye(128, dtype=np.float32) * np.float32(np.asarray(reg)))
    import ml_dtypes
    rgb = np.concatenate([np.eye(128, dtype=np.float32),
                          np.eye(128, dtype=np.float32)
                          * np.float32(np.asarray(reg))],
                         axis=1).astype(ml_dtypes.bfloat16)

    in_maps = []
    for c in range(N_CORES):
        sl = slice(c * BH_PER_CORE, (c + 1) * BH_PER_CORE)
        qt = np.ascontiguousarray(
            q[sl].transpose(0, 2, 1).reshape(BH_PER_CORE // 2, 2 * D, S)
        ).astype(np.float16)                                          # [2, 128, S]
        kt = np.ascontiguousarray(
            k[sl].transpose(0, 2, 1).reshape(BH_PER_CORE // 2, 2 * D, S)
        ).astype(np.float16)                                          # [2, 128, S]
        vv = v[sl].reshape(BH_PER_CORE, NT, 128, D)
        vs = np.ascontiguousarray(vv.transpose(0, 2, 1, 3)).reshape(
            BH_PER_CORE, 128, NT * D)                                 # [4, 128, 1024]
        in_maps.append({"qt": qt, "kt": kt, "vs": vs, "rg": rg, "rgb": rgb})
    return in_maps


def kernel(qry, key, val, reg):
    nc = build_nc()
    in_maps = _prep_inputs(qry, key, val, reg)
    res = run_bass_kernel_spmd(nc, in_maps, list(range(N_CORES)))
    out = np.empty((BH, S, D), dtype=np.float32)
    for c in range(N_CORES):
        ot = res.results[c]["ot"]                                    # [4, 64, S]
        for i in range(BH_PER_CORE):
            out[c * BH_PER_CORE + i] = ot[i].T
    return out.reshape(B, H, S, D)


# revision 38
# speedup vs baseline: 4.1137x; 1.3758x over previous
"""Trainium2 Bass kernel for nn_FAttention1d (attention with softmax over the
QUERY axis).

Reference computation (B=2, H=16, S=2048, D=64, fp32):
    att[b,h,q,k] = sum_d qry[b,h,q,d] * key[b,h,k,d]
    att += reg * I_S                      (diagonal in (q,k))
    att = softmax(att, axis=q)            (normalize over the QUERY axis)
    out[b,h,q,v] = sum_k att[b,h,q,k] * val[b,h,k,v]

Sharding: the 32 (b,h) pairs are split 4-per-core across 8 NeuronCores; the
whole S=2048 attention chain is local to a core.

Device-side layout: compute S^T = K @ Q^T with k on the partition axis, so the
q-axis softmax is a free-axis reduction (fused into the exp pass via the ACT
accumulator), and exp(S^T) tiles feed the A^T V matmul directly as the moving
operand:
    out^T[v,q] = sum_k (val[k,v] / r[k])_stationary @ exp(S^T)[k,q]
with r[k] = sum_q exp(S^T[k,q]) folded into the val rows.

Engine balance (per core ~16.8M exp elements move PSUM->SBUF):
  - ACT: all exp ops. A minority of (head, k-tile) score tiles ("direct") are
    exp'd straight from PSUM in two [128,1024] ops; the rest are staged to
    SBUF by DVE and exp'd in one wide [128,2048] op (cheaper per column for
    ACT, costs DVE a copy). The direct count balances ACT vs DVE.
  - DVE: staging copies, PSUM diag adds for direct tiles, reciprocal, out^T
    PSUM->SBUF copies.
  - Pool: diag adds on staged SBUF tiles, r half merges, val/r scaling.
  - PE: QK^T and AV matmuls, AV drained between QK chunks so PE follows the
    ACT/DVE pacemakers without bursts.
PSUM: out^T accumulator [128,2048] (4 banks) + 2x [128,1024] score tiles.
"""

import numpy as np
from collections import deque
from contextlib import ExitStack

import concourse.bass as bass
import concourse.mybir as mybir
import concourse.tile as tile
from concourse import bacc
from concourse.bass_utils import run_bass_kernel_spmd

B, H, S, D = 2, 16, 2048, 64
N_CORES = 8
BH = B * H                     # 32
BH_PER_CORE = BH // N_CORES    # 4
NT = S // 128                  # 16 k-tiles of 128
F32 = mybir.dt.float32
F16 = mybir.dt.float16
BF16 = mybir.dt.bfloat16

# Some (pair, k-tile, head) score tiles are exp'd directly from PSUM (2x
# [128,1024] ACT ops); the rest are DVE-staged to SBUF and exp'd in one
# [128,2048] ACT op (cheaper per column for ACT, costs DVE the copies).
# Direct tiles are head-A-only on every other k-tile: each PSUM score
# buffer's consumer is then a single ~1.2us op (one exp or one copy), which
# fits the reuse window of the two-buffer PSUM ping-pong — a fully-direct
# tile's 2.5us ACT drain stalls PE and starves DVE. 16 direct of 64
# balances ACT (~139us) against DVE (~126us) with Pool well under both.
DIRECT_NS = set()


def _is_direct(p, n, s):
    return s == 0 and n in DIRECT_NS


# "X" score tiles stage only the h0 half (one DVE copy); the h1 half is
# exp'd straight from PSUM. Both exps are [128,1024] ops, short enough to
# meet the PSUM ping-pong reuse window even behind one queued wide exp.
# Trades ~1.2us DVE for ~0.4us ACT per tile to balance the two pacemakers.
X_NS = set()


def _is_x(p, n, s):
    return n in X_NS


def _build_kernel(nc, tc, ctx, qt, kt, vs, rg, rgb, ot):
    const_pool = ctx.enter_context(tc.tile_pool(name="const", bufs=1))
    q_pool = ctx.enter_context(tc.tile_pool(name="q", bufs=2))
    k_pool = ctx.enter_context(tc.tile_pool(name="k", bufs=2))
    v_pool = ctx.enter_context(tc.tile_pool(name="v", bufs=2))
    e_pool = ctx.enter_context(tc.tile_pool(name="e", bufs=8))
    stg_pool = ctx.enter_context(tc.tile_pool(name="stg", bufs=3))
    r_pool = ctx.enter_context(tc.tile_pool(name="r", bufs=2))
    vsc_pool = ctx.enter_context(tc.tile_pool(name="vsc", bufs=6))
    osb_pool = ctx.enter_context(tc.tile_pool(name="osb", bufs=2))
    st_pool = ctx.enter_context(tc.tile_pool(name="st", bufs=2, space="PSUM"))
    o_pool = ctx.enter_context(tc.tile_pool(name="o", bufs=1, space="PSUM"))

    rg_eye = const_pool.tile([128, 128], F32)
    nc.sync.dma_start(rg_eye[:], rg[:])
    # [I, reg*I] as bf16 for the PE diag-accumulate matmul (I.T @ reg*I adds
    # reg to the score diagonal in PSUM, costing PE ~60ns instead of a DVE op)
    eyb = const_pool.tile([128, 256], BF16)
    nc.sync.dma_start(eyb[:], rgb[:])
    # warm the ACT Exp table during the input DMA so the first real exp
    # doesn't pay the 1.3us table load
    warm = const_pool.tile([128, 1], F32)
    nc.scalar.activation(warm[:], rg_eye[:, 0:1],
                         mybir.ActivationFunctionType.Exp)

    AB = (0, 1)
    for p in range(BH_PER_CORE // 2):
        bh = (2 * p, 2 * p + 1)
        q2 = q_pool.tile([128, S], F16, tag="q2", name="q2")
        k2 = k_pool.tile([128, S], F16, tag="k2", name="k2")
        # split input DMAs across both HWDGE queues so the first QK chunk
        # can start early
        nc.gpsimd.dma_start(k2[:, 0:128], kt[p][:, 0:128])
        nc.sync.dma_start(q2[:, 0:512], qt[p][:, 0:512])
        nc.sync.dma_start(q2[:, 512:1024], qt[p][:, 512:1024])
        nc.gpsimd.dma_start(k2[:, 128:], kt[p][:, 128:])
        nc.sync.dma_start(q2[:, 1024:], qt[p][:, 1024:])
        v_sb = [None, None]
        for s in AB:
            v_sb[s] = v_pool.tile([128, NT * 64], F32, tag=f"v{s}", name=f"v_sb{s}")
            nc.gpsimd.dma_start(v_sb[s][:], vs[bh[s]])

        # out^T for the pair: partitions 0-63 = bh A, 64-127 = bh B
        o_ps = o_pool.tile([128, S], F32)
        r_all = [r_pool.tile([128, 2, NT], F32, tag=f"rall{s}", name=f"r_all{s}") for s in AB]
        r_sum = [r_pool.tile([128, NT], F32, tag=f"rsum{s}", name=f"r_sum{s}") for s in AB]
        r_inv = [r_pool.tile([128, NT], F32, tag=f"rinv{s}", name=f"r_inv{s}") for s in AB]
        e_tiles = [[None] * NT, [None] * NT]
        vsc_tiles = [[None] * NT, [None] * NT]
        pending = deque()

        def queue_av_tiles(ms):
            # enqueue col-packed AV matmuls for k-tiles ms; drained a few at a
            # time between QK chunks so PE tracks the ACT/DVE pacemakers
            for m in ms:
                for ch in range(4):
                    pending.append((m, ch))

        out_sb = osb_pool.tile([128, S], F32)

        def drain_pending(k=2):
            for _ in range(k):
                if not pending:
                    return
                m, h = pending.popleft()
                ch = slice(h * 512, (h + 1) * 512)
                for s in AB:
                    # bh A -> out partitions 0-63, bh B -> 64-127
                    nc.tensor.matmul(
                        o_ps[64 * s:64 * s + 64, ch],
                        lhsT=vsc_tiles[s][m][:],
                        rhs=e_tiles[s][m][:, ch],
                        start=(m == 0),
                        stop=(m == NT - 1),
                        skip_group_check=True,
                    )
                if m == NT - 1:
                    # last accumulation for this q-chunk: evacuate + ship now
                    # so copies/DMAs overlap the remaining AV drains. The copy
                    # runs on ACT (same act table as Exp, no table switch) to
                    # keep DVE, the staging pacemaker, free.
                    nc.vector.tensor_copy(out_sb[:, ch], o_ps[:, ch])
                    for s in AB:
                        nc.gpsimd.dma_start(ot[bh[s]][:, ch],
                                            out_sb[64 * s:64 * s + 64, ch])

        def emit_vsc(ms):
            # r_inv on DVE (cheap [128,few] op); vsc[m] = val * r_inv[m] on
            # ACT as a Copy with a per-partition scale (same act table as
            # Exp, no table switch). Pool/gpsimd compute ops are avoided
            # entirely: their real Q7 launch overhead is micro-seconds, not
            # the ~100ns the cost model charges.
            if not ms:
                return
            for s in AB:
                nc.vector.reciprocal_approx_fast(
                    r_inv[s][:, ms[0]:ms[-1] + 1],
                    r_sum[s][:, ms[0]:ms[-1] + 1])
            for m in ms:
                for s in AB:
                    vsc = vsc_pool.tile([128, 64], BF16, tag=f"vsc{s}",
                                        name=f"vsc{s}")
                    vsc_tiles[s][m] = vsc
                    nc.vector.tensor_scalar_mul(
                        vsc[:], v_sb[s][:, m * 64:(m + 1) * 64],
                        r_inv[s][:, m:m + 1],
                    )

        # vsc for tile m is emitted inside tile VSC_AT[m] (after that tile's
        # diag units, so Pool never gates the diag->exp chain); its AV
        # matmuls are queued one tile later so PE never waits on fresh vsc.
        VSC_AT = {4: [0, 1, 2, 3], 8: [4, 5, 6, 7], 12: [8, 9, 10, 11],
                  14: [12], 15: [13]}
        QUEUE_AT = {5: [0, 1, 2, 3], 9: [4, 5, 6, 7], 13: [8, 9, 10, 11],
                    15: [12]}

        def emit_r_merge_and_vsc(ms):
            for m in ms:
                for s in AB:
                    if _is_direct(p, m, s) or _is_x(p, m, s):
                        nc.vector.tensor_add(
                            r_sum[s][:, m:m + 1], r_all[s][:, 0, m:m + 1],
                            r_all[s][:, 1, m:m + 1]
                        )
            emit_vsc(ms)

        for n in range(NT):
            hd = n // 8               # q-half containing this tile's diagonal
            cd = (n % 8) * 128        # diag column offset within that half
            queue_av_tiles(QUEUE_AT.get(n, []))
            for s in AB:
                e_tiles[s][n] = e_pool.tile([128, S], BF16, tag=f"e{s}",
                                            name=f"e{s}_{n}")
            stage = [None if _is_direct(p, n, s) else
                     stg_pool.tile(
                         [128, 1024 if _is_x(p, n, s) else S], F32,
                         tag=f"stg{s}" + ("x" if _is_x(p, n, s) else ""),
                         name=f"stage{s}")
                     for s in AB]
            for h in range(2):
                for s in AB:
                    direct = _is_direct(p, n, s)
                    st = st_pool.tile([128, 1024], F32)
                    for j in range(2):
                        q0 = h * 1024 + j * 512
                        nc.tensor.matmul(
                            st[:, j * 512:(j + 1) * 512],
                            lhsT=k2[64 * s:64 * s + 64, n * 128:(n + 1) * 128],
                            rhs=q2[64 * s:64 * s + 64, q0:q0 + 512],
                            start=True,
                            stop=True,
                        )
                    if direct:
                        if h == hd:
                            nc.vector.tensor_add(
                                st[:, cd:cd + 128], st[:, cd:cd + 128],
                                rg_eye[:])
                        nc.scalar.activation(
                            e_tiles[s][n][:, h * 1024:(h + 1) * 1024],
                            st[:],
                            mybir.ActivationFunctionType.Exp,
                            accum_out=r_all[s][:, h:h + 1, n:n + 1],
                        )
                    elif _is_x(p, n, s):
                        if h == 0:
                            nc.vector.tensor_copy(
                                stage[s][:, 0:1024], st[:])
                            if hd == 0:
                                nc.vector.tensor_add(
                                    stage[s][:, cd:cd + 128],
                                    stage[s][:, cd:cd + 128],
                                    rg_eye[:])
                        else:
                            if hd == 1:
                                nc.vector.tensor_add(
                                    st[:, cd:cd + 128], st[:, cd:cd + 128],
                                    rg_eye[:])
                            nc.scalar.activation(
                                e_tiles[s][n][:, 0:1024],
                                stage[s][:, 0:1024],
                                mybir.ActivationFunctionType.Exp,
                                accum_out=r_all[s][:, 0:1, n:n + 1],
                            )
                            nc.scalar.activation(
                                e_tiles[s][n][:, 1024:2048],
                                st[:],
                                mybir.ActivationFunctionType.Exp,
                                accum_out=r_all[s][:, 1:2, n:n + 1],
                            )
                    else:
                        if h == hd:
                            # diag add in PSUM on PE: st[:, cd:] += I.T @ reg*I
                            nc.tensor.matmul(
                                st[:, cd:cd + 128],
                                lhsT=eyb[:, 0:128],
                                rhs=eyb[:, 128:256],
                                start=False,
                                stop=True,
                                skip_group_check=True,
                            )
                        nc.vector.tensor_copy(
                            stage[s][:, h * 1024:(h + 1) * 1024], st[:])
                        if h == 1:
                            nc.scalar.activation(
                                e_tiles[s][n][:],
                                stage[s][:],
                                mybir.ActivationFunctionType.Exp,
                                accum_out=r_sum[s][:, n:n + 1],
                            )
                    if s == 1 and h == hd:
                        emit_r_merge_and_vsc(VSC_AT.get(n, []))
                    drain_pending(1 if n < 6 else (2 if n < 13 else 3))
        emit_r_merge_and_vsc([14, 15])
        queue_av_tiles([13, 14, 15])
        while pending:
            drain_pending(4)


_NC_CACHE = {}


def build_nc(repeats=1):
    key = repeats
    if key in _NC_CACHE:
        return _NC_CACHE[key]
    nc = bacc.Bacc("TRN2", target_bir_lowering=False, debug=False)
    qt = nc.dram_tensor("qt", [BH_PER_CORE // 2, 2 * D, S], F16, kind="ExternalInput").ap()
    kt = nc.dram_tensor("kt", [BH_PER_CORE // 2, 2 * D, S], F16, kind="ExternalInput").ap()
    vs = nc.dram_tensor("vs", [BH_PER_CORE, 128, NT * 64], F32, kind="ExternalInput").ap()
    rg = nc.dram_tensor("rg", [128, 128], F32, kind="ExternalInput").ap()
    rgb = nc.dram_tensor("rgb", [128, 256], BF16, kind="ExternalInput").ap()
    ot = nc.dram_tensor("ot", [BH_PER_CORE, D, S], F32, kind="ExternalOutput").ap()
    with tile.TileContext(nc) as tc, ExitStack() as ctx:
        if repeats == 1:
            _build_kernel(nc, tc, ctx, qt, kt, vs, rg, rgb, ot)
        else:
            # benchmarking mode: repeat the whole kernel body in an on-device
            # loop so per-iteration time can be extracted from wall clock
            with tc.For_i(0, repeats, 1,
                          hint_engines=(mybir.EngineType.PE,
                                        mybir.EngineType.Activation,
                                        mybir.EngineType.DVE)):
                _build_kernel(nc, tc, ctx, qt, kt, vs, rg, rgb, ot)
    nc.compile()
    _NC_CACHE[key] = nc
    return nc


def _prep_inputs(qry, key, val, reg):
    """Host-side shard + layout prep. Returns per-core input maps."""
    q = np.ascontiguousarray(np.asarray(qry, dtype=np.float32)).reshape(BH, S, D)
    k = np.ascontiguousarray(np.asarray(key, dtype=np.float32)).reshape(BH, S, D)
    v = np.ascontiguousarray(np.asarray(val, dtype=np.float32)).reshape(BH, S, D)
    rg = (np.eye(128, dtype=np.float32) * np.float32(np.asarray(reg)))
    import ml_dtypes
    rgb = np.concatenate([np.eye(128, dtype=np.float32),
                          np.eye(128, dtype=np.float32)
                          * np.float32(np.asarray(reg))],
                         axis=1).astype(ml_dtypes.bfloat16)

    in_maps = []
    for c in range(N_CORES):
        sl = slice(c * BH_PER_CORE, (c + 1) * BH_PER_CORE)
        qt = np.ascontiguousarray(
            q[sl].transpose(0, 2, 1).reshape(BH_PER_CORE // 2, 2 * D, S)
        ).astype(np.float16)                                          # [2, 128, S]
        kt = np.ascontiguousarray(
            k[sl].transpose(0, 2, 1).reshape(BH_PER_CORE // 2, 2 * D, S)
        ).astype(np.float16)                                          # [2, 128, S]
        vv = v[sl].reshape(BH_PER_CORE, NT, 128, D)
        vs = np.ascontiguousarray(vv.transpose(0, 2, 1, 3)).reshape(
            BH_PER_CORE, 128, NT * D)                                 # [4, 128, 1024]
        in_maps.append({"qt": qt, "kt": kt, "vs": vs, "rg": rg, "rgb": rgb})
    return in_maps


def kernel(qry, key, val, reg):
    nc = build_nc()
    in_maps = _prep_inputs(qry, key, val, reg)
    res = run_bass_kernel_spmd(nc, in_maps, list(range(N_CORES)))
    out = np.empty((BH, S, D), dtype=np.float32)
    for c in range(N_CORES):
        ot = res.results[c]["ot"]                                    # [4, 64, S]
        for i in range(BH_PER_CORE):
            out[c * BH_PER_CORE + i] = ot[i].T
    return out.reshape(B, H, S, D)
